# revision 65
# baseline (speedup 1.0000x reference)
"""Trainium2 Bass kernel for nn_Metric_42674795053594 (Relation Network loss).

Self-contained: hardcodes all shapes. Shards batch b=8 across 8 NeuronCores
(1 episode/core), replicates params, uses AllReduce for training-mode
BatchNorm statistics that couple all episodes.
"""
import sys, os
sys.path.insert(0, '/opt/trn_rl_repo')
import numpy as np
import ml_dtypes

import concourse.bass as bass
import concourse.mybir as mybir
import concourse.tile as tile
from concourse import bacc
from concourse.bass_utils import run_bass_kernel_spmd

F32 = mybir.dt.float32
F32R = mybir.dt.float32r   # 1 cycle/row on PE when out width >= 256 (vs 4 for f32)
BF16 = mybir.dt.bfloat16
AF = mybir.ActivationFunctionType
ALU = mybir.AluOpType
AX = mybir.AxisListType

EPS = 1e-5
NCORES = 8
S, Q = 5, 30
NPAIR = 18          # 36 image slots (5 sup + 30 qry + 1 pad) packed 2/partition-half
IMGW = 84
PLANE = 7232        # padded per-channel plane stride (>= 7056 + 170)
W1 = 7056           # conv1 output width (84*84)
PW1, PW2 = 1681, 361   # pooled widths: 41*41, 19*19
PAD1, PAD2 = 88, 40    # shift-overshoot pads (2*41+2+chunk slack, 2*19+2)
GROWS = Q * S * 81     # 12150 pairwise rows per core
CHUNK = 486            # 6 blocks of 81, <= 512

# conv2 input pooled1 is stored in this dtype (bf16 halves SBUF + 2x PE for conv2)
POOL1_DT = BF16


def _bn_scalar_ops2(nc, pool, s_ap, q_ap, invn_ap, g2_ap, b2_ap, sc_out, sh_out, eps_ap, tag):
    """Batched: all tensor args are [P,2] (two BN regions processed at once)."""
    P = s_ap.shape[0]
    t = pool.tile([128, 8], F32, tag=f"bns2_{tag}")
    mean, ex2, var, m2 = t[:P, 0:2], t[:P, 2:4], t[:P, 4:6], t[:P, 6:8]
    nc.vector.tensor_tensor(mean, s_ap, invn_ap, ALU.mult)
    nc.vector.tensor_tensor(ex2, q_ap, invn_ap, ALU.mult)
    nc.vector.tensor_tensor(m2, mean, mean, ALU.mult)
    nc.vector.tensor_tensor(var, ex2, m2, ALU.subtract)
    nc.scalar.activation(var, var, AF.Sqrt, bias=eps_ap)
    nc.vector.reciprocal(var, var)
    nc.vector.tensor_tensor(sc_out, g2_ap, var, ALU.mult)
    nc.vector.tensor_tensor(m2, mean, sc_out, ALU.mult)
    nc.vector.tensor_tensor(sh_out, b2_ap, m2, ALU.subtract)


def _bn_scalar_ops(nc, pool, s_ap, q_ap, n_elems, g_ap, b_ap, sc_out, sh_out, eps_ap, tag):
    """Given sum (s_ap) and sumsq (q_ap) APs [P,1], counts, gamma/beta APs,
    write scale into sc_out and shift into sh_out ([P,1])."""
    P = s_ap.shape[0]
    t = pool.tile([128, 4], F32, tag=f"bns_{tag}")
    mean, ex2, var, m2 = t[:P, 0:1], t[:P, 1:2], t[:P, 2:3], t[:P, 3:4]
    nc.vector.tensor_scalar_mul(mean, s_ap, 1.0 / n_elems)
    nc.vector.tensor_scalar_mul(ex2, q_ap, 1.0 / n_elems)
    nc.vector.tensor_tensor(m2, mean, mean, ALU.mult)
    nc.vector.tensor_tensor(var, ex2, m2, ALU.subtract)
    # sd = sqrt(var + eps); inv = 1/sd
    nc.scalar.activation(var, var, AF.Sqrt, bias=eps_ap)
    nc.vector.reciprocal(var, var)
    nc.vector.tensor_tensor(sc_out, g_ap, var, ALU.mult)
    nc.vector.tensor_tensor(m2, mean, sc_out, ALU.mult)
    nc.vector.tensor_tensor(sh_out, b_ap, m2, ALU.subtract)


def build_nc(n_cores=NCORES, debug=False):
    nc = bacc.Bacc("TRN2", target_bir_lowering=False, debug=False, num_devices=n_cores)
    RG = [list(range(n_cores))]

    # ---------------- I/O ----------------
    imgs_d = nc.dram_tensor("imgs", [36, 3, PLANE], BF16, kind="ExternalInput")
    w1t_d = nc.dram_tensor("w1t", [54, 128], BF16, kind="ExternalInput")
    wct_d = nc.dram_tensor("wct", [128, 3, 9, 128], F32R, kind="ExternalInput")
    bnx_d = nc.dram_tensor("bnx", [64, 8], F32, kind="ExternalInput")
    gw1_d = nc.dram_tensor("gw1", [66, 512], F32, kind="ExternalInput")
    wbf_d = nc.dram_tensor("wbf", [128, 2688], BF16, kind="ExternalInput")
    pk128_d = nc.dram_tensor("pk128", [128, 16], F32, kind="ExternalInput")
    fb3_d = nc.dram_tensor("fb3t", [64, 1], F32, kind="ExternalInput")
    fw4_d = nc.dram_tensor("fw4t", [64, 1], BF16, kind="ExternalInput")
    fb4_d = nc.dram_tensor("fb4t", [1, 1], F32, kind="ExternalInput")
    coord_d = nc.dram_tensor("coordp", [2, 315], F32, kind="ExternalInput")
    lap_d = nc.dram_tensor("lap", [1, 300], F32, kind="ExternalInput")

    loss_d = nc.dram_tensor("loss_part", [1, 1], F32, kind="ExternalOutput")
    if debug:
        feats_dbg_d = nc.dram_tensor("feats_dbg", [66, 324], F32, kind="ExternalOutput")
        dist_dbg_d = nc.dram_tensor("dist_dbg", [1, 150], F32, kind="ExternalOutput")
        xf_dbg_d = nc.dram_tensor("xf_dbg", [128, 2, 150], F32, kind="ExternalOutput")

    with tile.TileContext(nc) as tc:
        with (
            tc.tile_pool(name="pers", bufs=1) as pers,
            tc.tile_pool(name="dram", bufs=1, space="DRAM") as dram,
        ):
            # ---------------- load persistent weights ----------------
            w1t = pers.tile([54, 128], BF16)
            nc.sync.dma_start(w1t[:], w1t_d[:])
            wct = pers.tile([128, 3, 9, 128], F32R)
            nc.sync.dma_start(wct[:], wct_d[:])
            wct2b = pers.tile([128, 9, 128], POOL1_DT)   # conv2 weights in pooled1 dtype
            nc.vector.tensor_copy(wct2b[:], wct[:, 0])
            bnx = pers.tile([128, 8], F32)
            nc.sync.dma_start(bnx[0:64, :], bnx_d[:])
            nc.sync.dma_start(bnx[64:128, :], bnx_d[:])
            bng = bnx[:, 0:4]
            bnb = bnx[:, 4:8]
            gw1 = pers.tile([66, 512], F32)
            nc.sync.dma_start(gw1[:], gw1_d[:])
            gw1s = gw1[:, 0:256]
            gw1q = gw1[:, 256:512]
            wbf = pers.tile([128, 2688], BF16)
            nc.sync.dma_start(wbf[:], wbf_d[:])
            gwt = wbf[:, 0:1536].rearrange("p (l k m) -> p l k m", l=3, k=2)
            fwt = wbf[:, 1536:2560].rearrange("p (l k m) -> p l k m", l=2, k=2)
            fw3 = wbf[:, 2560:2688].rearrange("p (k m) -> p k m", k=2)
            pk128 = pers.tile([128, 16], F32)
            nc.sync.dma_start(pk128[:], pk128_d[:])
            gb1 = pk128[:, 0:2]
            gbt = pk128[:, 2:8].rearrange("p (l m) -> p l m", m=2)
            fbt = pk128[:, 8:12].rearrange("p (l m) -> p l m", m=2)
            fbng = pk128[:, 12:14]
            fbnb = pk128[:, 14:16]
            fb3 = pers.tile([64, 1], F32)
            nc.sync.dma_start(fb3[:], fb3_d[:])
            fw4 = pers.tile([64, 1], BF16)
            nc.sync.dma_start(fw4[:], fw4_d[:])
            fb4 = pers.tile([1, 1], F32)
            nc.sync.dma_start(fb4[:], fb4_d[:])
            lap = pers.tile([1, 300], F32)
            nc.sync.dma_start(lap[:], lap_d[:])
            lbl_sb = lap[:, 0:150]
            apmask_sb = lap[:, 150:300]

            epsc = pers.tile([128, 1], F32)
            nc.gpsimd.memset(epsc[:], EPS)
            # first Act op is a Sqrt so the sqrt act-table set loads during
            # startup instead of on the BN1 critical path
            warmt = pers.tile([1, 1], F32)
            nc.scalar.activation(warmt[:], epsc[0:1, 0:1], AF.Sqrt, bias=epsc[0:1, 0:1])
            margin = pers.tile([1, 1], F32)
            nc.gpsimd.memset(margin[:], 0.2)

            # persistent activations
            pooled2 = pers.tile([128, NPAIR * PW2 + PAD2], F32R)
            nc.gpsimd.memset(pooled2[:, NPAIR * PW2:].bitcast(F32), 0.0)
            feats = pers.tile([66, 324], F32)
            nc.sync.dma_start(feats[64:66, 0:315], coord_d[:])
            # BN scale/shift per conv layer: [128, 2] (col0 sup, col1 qry)
            sc_t = [pers.tile([128, 2], F32, tag=f"sc{l}", name=f"sc{l}") for l in range(4)]
            sh_t = [pers.tile([128, 2], F32, tag=f"sh{l}", name=f"sh{l}") for l in range(4)]
            scmix = [pers.tile([128, 1], F32, tag=f"scm{l}", name=f"scm{l}") for l in range(4)]
            shmix = [pers.tile([128, 1], F32, tag=f"shm{l}", name=f"shm{l}") for l in range(4)]
            xf = pers.tile([128, 2, 150], F32)

            # ---- helper: stats + allreduce + scale/shift for one conv layer ----
            def conv_bn(layer, buf, Wimg, valid_view_fn, sup_elems, qry_elems, cc_tag,
                        sum_axis=AX.X, sum_on_act=False):
                """buf: [128, NPAIR*Wimg(+pad)]; valid_view_fn(half_slice, p0, np_)
                returns the valid-region AP for pairs [p0, p0+np_).
                Computes region sums + per-pair sumsq, allreduces, fills
                sc_t[layer], sh_t[layer]."""
                st = pers.tile([128, 8], F32, tag=f"stt{layer}")
                # bottom-half sup columns are never written by the region
                # reduces; zero them so the full [128,4] block can ride the
                # collective and the halves fold in during the gather-reduce
                nc.gpsimd.memset(st[64:128, 0:1], 0.0)
                nc.gpsimd.memset(st[64:128, 2:3], 0.0)
                # per-pair sums + sumsq (overlap with conv); region-combine later
                sumacc = pers.tile([128, NPAIR], F32, tag=f"sma{layer}")
                sqacc = pers.tile([128, NPAIR], F32, tag=f"sqa{layer}")
                sqs = pers.tile([128, 512], F32, tag="sq_scratch")
                if sum_on_act:
                    # full-width bf16 dump scratches; Act accum_out yields the sums
                    # directly, freeing DVE for the maxpool
                    cpd = pers.tile([128, Wimg], BF16, tag="cp_dump")
                    sqd = pers.tile([128, Wimg], BF16, tag="sq_dump")
                for p in range(NPAIR):
                    v = valid_view_fn(slice(0, 128), p, 1)
                    n_el = v.free_size()
                    if sum_on_act:
                        if p == NPAIR - 1:
                            # tail pair: sum on DVE so it overlaps Act's Square
                            # instead of serializing 3.2us of Act at phase end
                            nc.vector.reduce_sum(sumacc[:, p:p + 1], v, axis=sum_axis)
                        else:
                            nc.scalar.activation(cpd[:, :n_el], v, AF.Copy,
                                                 accum_out=sumacc[:, p:p + 1])
                        nc.scalar.activation(sqd[:, :n_el], v, AF.Square,
                                             accum_out=sqacc[:, p:p + 1])
                        continue
                    nc.vector.reduce_sum(sumacc[:, p:p + 1], v, axis=sum_axis)
                    # chunk the square dump through the 512-wide scratch
                    if n_el <= 512:
                        nc.scalar.activation(sqs[:, :n_el], v, AF.Square,
                                             accum_out=sqacc[:, p:p + 1])
                    else:
                        # pooled maps are contiguous; square in 512-col chunks
                        flat = buf[:, p * Wimg:(p + 1) * Wimg]
                        nsub = (Wimg + 511) // 512
                        part = pers.tile([128, 4], F32, tag=f"sqp{layer}")
                        for sub in range(nsub):
                            a, b = sub * 512, min((sub + 1) * 512, Wimg)
                            nc.scalar.activation(sqs[:, :b - a], flat[:, a:b], AF.Square,
                                                 accum_out=part[:, sub % 4:sub % 4 + 1])
                        # sum the partials (nsub<=4)
                        nc.vector.reduce_sum(sqacc[:, p:p + 1], part[:, :nsub], axis=AX.X)
                nc.vector.reduce_sum(st[0:64, 0:1], sumacc[0:64, 0:5], axis=AX.X)
                nc.vector.reduce_sum(st[0:64, 1:2], sumacc[0:64, 5:18], axis=AX.X)
                nc.vector.reduce_sum(st[64:128, 1:2], sumacc[64:128, :], axis=AX.X)
                nc.vector.reduce_sum(st[0:64, 2:3], sqacc[0:64, 0:5], axis=AX.X)
                nc.vector.reduce_sum(st[0:64, 3:4], sqacc[0:64, 5:18], axis=AX.X)
                nc.vector.reduce_sum(st[64:128, 3:4], sqacc[64:128, :], axis=AX.X)
                # collective carries the full [128,4]: (sup_sum, qry_sum,
                # sup_sq, qry_sq) with qry split across halves and the (zeroed)
                # bottom sup columns folding away in the gather-reduce
                bin_ = dram.tile([128, 4], F32, tag=f"ccin{cc_tag}")
                bout = dram.tile([128 * n_cores, 4], F32, tag=f"ccout{cc_tag}")
                nc.sync.dma_start(bin_[:], st[:, 0:4])
                nc.gpsimd.collective_compute("AllGather", ALU.bypass, replica_groups=RG,
                                             ins=[bin_.opt()], outs=[bout.opt()])
                gat = pers.tile([128, 8 * n_cores], F32, tag=f"gat{layer}")
                gsrc = bout.rearrange("(r h p) f -> p (r h) f", h=2, p=64)
                nc.sync.dma_start(gat[0:64, :], gsrc)
                nc.sync.dma_start(gat[64:128, :], gsrc)
                red = pers.tile([128, 4], F32, tag=f"red{layer}")
                nc.vector.reduce_sum(red[:], gat.rearrange("p (r f) -> p f r", r=2 * n_cores),
                                     axis=AX.X)
                _bn_scalar_ops(nc, pers, red[:, 0:1], red[:, 2:3], sup_elems,
                               bng[:, layer:layer + 1], bnb[:, layer:layer + 1],
                               sc_t[layer][:, 0:1], sh_t[layer][:, 0:1], epsc[:], f"s{layer}")
                _bn_scalar_ops(nc, pers, red[:, 1:2], red[:, 3:4], qry_elems,
                               bng[:, layer:layer + 1], bnb[:, layer:layer + 1],
                               sc_t[layer][:, 1:2], sh_t[layer][:, 1:2], epsc[:], f"q{layer}")
                # mixed vectors for pairs 0-4 (sup scale on top, qry on bottom) so
                # BN+relu is ONE Act instruction per pair (cost is free-size only)
                nc.vector.tensor_copy(scmix[layer][0:64], sc_t[layer][0:64, 0:1])
                nc.vector.tensor_copy(scmix[layer][64:128], sc_t[layer][64:128, 1:2])
                nc.vector.tensor_copy(shmix[layer][0:64], sh_t[layer][0:64, 0:1])
                nc.vector.tensor_copy(shmix[layer][64:128], sh_t[layer][64:128, 1:2])

            def bn_apply_pairs(layer, view_fn, out_view_fn=None, chunk_first=False):
                # per-pair BN+relu so the next conv layer pipelines behind it
                for p in range(NPAIR):
                    full_in = view_fn(slice(0, 128), p, 1)
                    full_out = out_view_fn(slice(0, 128), p, 1) if out_view_fn else full_in
                    if p < 5:
                        sc, sh = scmix[layer][:], shmix[layer][:]
                    else:
                        sc, sh = sc_t[layer][:, 1:2], sh_t[layer][:, 1:2]
                    if chunk_first and p == 0:
                        # chunked so the next conv's first matmul starts after
                        # the first third instead of the full 1.5us apply
                        for a, b in ((0, 561), (561, 1122), (1122, 1681)):
                            nc.scalar.activation(full_out[:, a:b], full_in[:, a:b],
                                                 AF.Relu, bias=sh, scale=sc)
                    else:
                        nc.scalar.activation(full_out, full_in, AF.Relu, bias=sh, scale=sc)

            # ================= PHASE 1: conv1 + pool + BN, conv2 + pool + BN =================
            with (
                tc.tile_pool(name="ph1", bufs=1) as ph1,
                tc.tile_pool(name="ph1b", bufs=4) as ph1b,
                tc.tile_pool(name="ph1c", bufs=8) as ph1c,
                tc.tile_pool(name="ph1ps", bufs=8, space="PSUM") as psum,
            ):
                pooled1 = ph1.tile([128, NPAIR * PW1 + PAD1], POOL1_DT)
                nc.gpsimd.memset(pooled1[:, NPAIR * PW1:], 0.0)

                for p in range(NPAIR):
                    in27 = ph1b.tile([54, W1], BF16, tag="in27")
                    for half, img in ((0, p), (1, 18 + p)):
                        for kx in range(3):
                            src_ap = bass.AP(tensor=imgs_d.ap().tensor,
                                             offset=img * 3 * PLANE + kx * IMGW,
                                             ap=[[1, 3], [PLANE, 3], [1, W1]])
                            r0 = half * 27 + kx * 9
                            nc.sync.dma_start(in27[r0:r0 + 9, :], src_ap)
                    # 14 chunks of 504 cols = 6 input rows each; pool 2x2 from PSUM.
                    # PSUM can only be read by one non-scalar input, so the pool is
                    # either a single DVE tensor_reduce, or (for a subset of chunks)
                    # an Act bf16 copy to SBUF + 2-stage Pool-engine max — spreading
                    # the PSUM-evacuation load across all three engines.
                    deferred = []
                    for c in range(14):
                        a = c * 504
                        w = 504 if c < 13 else 336
                        inr = 6 if c < 13 else 4
                        ps = psum.tile([128, 512], F32, tag="cps")
                        nc.tensor.matmul(ps[:, :w], w1t[:, :], in27[:, a:a + w])
                        orows = 3 if c < 13 else 2
                        dst = pooled1[:, p * PW1 + 3 * c * 41: p * PW1 + (3 * c + orows) * 41]
                        if False:
                            # Act evacuates PSUM (bf16 copy, frees the bank); DVE's
                            # 2-stage bf16 max is DEFERRED to the end of the pair so
                            # the DVE queue never head-of-line blocks on an Act copy
                            cp = ph1c.tile([128, 512], POOL1_DT, tag="cpscr")
                            nc.scalar.activation(cp[:, :w], ps[:, :w], AF.Copy)
                            deferred.append((cp, dst, inr, orows, w))
                        else:
                            v5 = ps[:, :w].rearrange("p (r c) -> p r c", r=inr)[:, :, 0:82] \
                                .rearrange("p (R rp) (C cp) -> p R C rp cp", rp=2, cp=2)[:, :orows]
                            nc.vector.tensor_reduce(dst.rearrange("p (r c) -> p r c", r=orows),
                                                    v5, axis=AX.XY, op=ALU.max)
                    for cp, dst, inr, orows, w in deferred:
                        ve = cp[:, :w].rearrange("p (r C two) -> p r C two", two=2, r=inr)
                        s1 = ph1b.tile([128, 256], POOL1_DT, tag="s1scr")
                        s1v = s1[:, :inr * 41].rearrange("p (r c) -> p r c", r=inr)
                        nc.vector.tensor_tensor(s1v, ve[:, :, 0:41, 0],
                                                ve[:, :, 0:41, 1], ALU.max)
                        s2 = s1[:, :inr * 41].rearrange("p (R two c) -> p R two c",
                                                        two=2, c=41)
                        nc.vector.tensor_tensor(dst.rearrange("p (r c) -> p r c", r=orows),
                                                s2[:, :, 0, :], s2[:, :, 1, :], ALU.max)

                # ---- L1 BN ----
                def l1_view(hs, p0, np_):
                    return pooled1[hs, p0 * PW1:(p0 + np_) * PW1]
                conv_bn(0, pooled1, PW1, l1_view, NCORES * 5 * PW1, NCORES * 30 * PW1, 0,
                        sum_on_act=True)
                bn_apply_pairs(0, l1_view, chunk_first=True)
                nc.gpsimd.memset(pooled1[64:128, 17 * PW1:18 * PW1], 0.0)  # pad img

                # ---- conv2 (bf16 in, fp32 psum) + pool (41->39 valid ->19) ----
                c2widths = [492, 492, 492, 82]
                for p in range(NPAIR):
                    base = p * PW1
                    pstiles = [psum.tile([128, 512], F32, tag="cps", name=f"c2ps{_i}") for _i in range(4)]
                    for j in range(9):
                        sh = (j // 3) * 41 + (j % 3)
                        for c in range(4):
                            a = c * 492
                            w = c2widths[c]
                            nc.tensor.matmul(
                                pstiles[c][:, :w], wct2b[:, j, :],
                                pooled1[:, base + a + sh: base + a + sh + w],
                                start=(j == 0), stop=(j == 8))
                    for c in range(4):
                        orows = 6 if c < 3 else 1
                        inrows = 12 if c < 3 else 2
                        v5 = pstiles[c][:, :inrows * 41].rearrange("p (r c) -> p r c", r=inrows)[:, :2 * orows, 0:38] \
                            .rearrange("p (R rp) (C cp) -> p R C rp cp", rp=2, cp=2)
                        dst = pooled2[:, p * PW2 + 6 * c * 19: p * PW2 + (6 * c + orows) * 19]
                        nc.vector.tensor_reduce(dst.rearrange("p (r c) -> p r c", r=orows),
                                                v5, axis=AX.XY, op=ALU.max)

            # ---- L2 BN ----
            def l2_view(hs, p0, np_):
                return pooled2[hs, p0 * PW2:(p0 + np_) * PW2]
            conv_bn(1, pooled2, PW2, l2_view, NCORES * 5 * PW2, NCORES * 30 * PW2, 1)
            bn_apply_pairs(1, l2_view)
            nc.gpsimd.memset(pooled2[64:128, 17 * PW2:18 * PW2].bitcast(F32), 0.0)

            # ================= PHASE 2: conv3, conv4, avgpool =================
            with (
                tc.tile_pool(name="ph2", bufs=1) as ph2,
                tc.tile_pool(name="ph2ps", bufs=8, space="PSUM") as psum,
            ):
                c3buf = ph2.tile([128, NPAIR * PW2 + PAD2], F32)
                nc.gpsimd.memset(c3buf[:, NPAIR * PW2:], 0.0)
                PW3 = 289  # 17*17 repacked width for conv4
                c17 = ph2.tile([128, NPAIR * PW3 + 36], F32R)
                nc.gpsimd.memset(c17[:, NPAIR * PW3:].bitcast(F32), 0.0)
                c4buf = ph2.tile([128, NPAIR * PW3], F32)

                def conv_layer(src, dstbuf, lidx, W, Wo, Wc):
                    # src [128, NPAIR*W(+pad)]; dst stride Wo; compute only Wc cols
                    # fp32r matmuls: 1 cycle/row (Wc >= 256) with ~tf32 precision
                    for pb in range(0, NPAIR, 4):
                        pe = min(pb + 4, NPAIR)
                        pst = {pp: psum.tile([128, 512], F32, tag="cps", name=f"c34ps{pp}") for pp in range(pb, pe)}
                        rowlen = int(round(W ** 0.5))
                        for j in range(9):
                            sh = (j // 3) * rowlen + (j % 3)
                            for pp in range(pb, pe):
                                base = pp * W
                                nc.tensor.matmul(
                                    pst[pp][:, :Wc], wct[:, lidx, j, :].bitcast(F32R),
                                    src[:, base + sh: base + sh + Wc].bitcast(F32R),
                                    start=(j == 0), stop=(j == 8))
                        for pp in range(pb, pe):
                            nc.scalar.activation(dstbuf[:, pp * Wo:pp * Wo + Wc],
                                                 pst[pp][:, :Wc], AF.Copy)

                conv_layer(pooled2, c3buf, 1, PW2, PW2, 324)  # 17*19=323 needed; even for f32r

                def l3_view(hs, p0, np_):
                    return c3buf[hs, p0 * PW2:(p0 + np_) * PW2].rearrange(
                        "p (i r c) -> p i r c", r=19, c=19)[:, :, 0:17, 0:17]
                def c17_view(hs, p0, np_):
                    return c17[hs, p0 * PW3:(p0 + np_) * PW3].rearrange(
                        "p (i r c) -> p i r c", r=17, c=17)
                conv_bn(2, c3buf, PW2, l3_view, NCORES * 5 * 289, NCORES * 30 * 289, 2, sum_axis=AX.XYZ)
                bn_apply_pairs(2, l3_view, c17_view)
                nc.gpsimd.memset(c17[64:128, 17 * PW3:18 * PW3].bitcast(F32), 0.0)

                conv_layer(c17, c4buf, 2, PW3, PW3, 256)  # 255 needed; 256 for f32r fast path

                def l4_view(hs, p0, np_):
                    return c4buf[hs, p0 * PW3:(p0 + np_) * PW3].rearrange(
                        "p (i r c) -> p i r c", r=17, c=17)[:, :, 0:15, 0:15]
                conv_bn(3, c4buf, PW3, l4_view, NCORES * 5 * 225, NCORES * 30 * 225, 3, sum_axis=AX.XYZ)
                bn_apply_pairs(3, l4_view)

                # ---- avgpool 5x5 -> [64, 9] per image ----
                featsB = ph2.tile([128, 162], F32)
                ptmp = ph2.tile([128, 90], F32, tag="ptmp")
                # two images per reduce (4-dim AP limit allows [i, r, oc, k]) to
                # halve the serial DVE instruction stream gating phase 4
                groups = [(0, p, 2) for p in range(0, 18, 2)]                     + [(1, p, 2) for p in range(0, 16, 2)] + [(1, 16, 1)]
                for half, p, ni in groups:
                    hs = slice(half * 64, half * 64 + 64)
                    base = p * PW3
                    v1 = c4buf[hs, base:base + ni * PW3].rearrange(
                        "p (i r c) -> p i r c", i=ni, r=17)[:, :, 0:15, 0:15].rearrange(
                        "p i r (oc k) -> p i r oc k", oc=3)
                    nc.vector.reduce_sum(
                        ptmp[hs, :ni * 45].rearrange("p (i r oc) -> p i r oc", i=ni, r=15),
                        v1, axis=AX.X)
                    v2 = ptmp[hs, :ni * 45].rearrange("p (i R k oc) -> p i R oc k",
                                                      i=ni, R=3, k=5, oc=3)
                    if half == 0:
                        dst = feats[0:64, p * 9:(p + ni) * 9].rearrange(
                            "p (i R oc) -> p i R oc", i=ni, R=3)
                        nc.vector.reduce_sum(dst, v2, axis=AX.X)
                    else:
                        dstB = featsB[hs, p * 9:(p + ni) * 9].rearrange(
                            "p (i R oc) -> p i R oc", i=ni, R=3)
                        nc.vector.reduce_sum(dstB, v2, axis=AX.X)
                nc.sync.dma_start(feats[0:64, 162:315], featsB[64:128, 0:153])
                nc.vector.tensor_scalar_mul(feats[0:64, 0:315], feats[0:64, 0:315], 1.0 / 25.0)

            if debug:
                nc.sync.dma_start(feats_dbg_d[:], feats[:])

            # ================= PHASE 3: pairwise g-MLP + f-MLP + loss =================
            with (
                tc.tile_pool(name="ph3", bufs=5) as ph3,
                tc.tile_pool(name="ph3psg", bufs=4, space="PSUM") as psg,
            ):
                # A[mb] [128, 45], B[mb] [128, 270]
                A = [ph3.tile([128, 45], BF16, tag=f"A{m}", name=f"A{m}") for m in range(2)]
                Bq = [ph3.tile([128, 270], BF16, tag=f"B{m}", name=f"B{m}") for m in range(2)]
                for m in range(2):
                    pa = psg.tile([128, 512], F32, tag="gps0")
                    nc.tensor.matmul(pa[:, 0:45], gw1s[:, m * 128:(m + 1) * 128], feats[:, 0:45])
                    nc.scalar.activation(A[m][:], pa[:, 0:45], AF.Identity, bias=gb1[:, m:m + 1])
                    pb = psg.tile([128, 512], F32, tag="gps1")
                    nc.tensor.matmul(pb[:, 0:270], gw1q[:, m * 128:(m + 1) * 128], feats[:, 45:315])
                    nc.scalar.activation(Bq[m][:], pb[:, 0:270], AF.Copy)

                QCH = 405  # one query row-block: 5 s * 81 xy
                for qp in range(0, Q, 2):
                    qpair = (qp, qp + 1)
                    h = {}
                    for qi, q in enumerate(qpair):
                        x1 = [ph3.tile([128, QCH], BF16, tag=f"x1_{qi}_{k}", name=f"x1_{qi}_{k}")
                              for k in range(2)]
                        for k in range(2):
                            a_in = A[k][:, :, None].to_broadcast((128, 45, 9))
                            b_in = Bq[k][:, None, q * 9:q * 9 + 9].to_broadcast((128, 45, 9))
                            out = x1[k][:].rearrange("p (sx y) -> p sx y", y=9)
                            eng = nc.vector if k == 0 else nc.gpsimd
                            eng.tensor_tensor(out, a_in, b_in, ALU.add)
                            eng.tensor_scalar_max(x1[k][:], x1[k][:], 0.0)
                        h[qi] = x1
                    for l in range(3):
                        hn = {qi: [ph3.tile([128, QCH], BF16, tag=f"h{qi}_{l}_{m}", name=f"h{qi}_{l}_{m}")
                                   for m in range(2)] for qi in range(2)}
                        for m in range(2):
                            ps = {qi: psg.tile([128, 512], F32, tag=f"gps{qi}", name=f"gps{qi}")
                                  for qi in range(2)}
                            for ks in range(2):
                                for qi in range(2):
                                    nc.tensor.matmul(ps[qi][:, :QCH],
                                                     gwt[:, l, ks, m * 128:(m + 1) * 128],
                                                     h[qi][ks][:],
                                                     start=(ks == 0), stop=(ks == 1))
                            for qi in range(2):
                                # spread relu+bias: 3 to DVE, 9 to Act (GPSIMD
                                # cannot touch PSUM)
                                if m == 1 and qi == 1:
                                    nc.vector.tensor_scalar(hn[qi][m][:], ps[qi][:, :QCH],
                                                            gbt[:, l, m:m + 1], 0.0,
                                                            ALU.add, ALU.max)
                                else:
                                    nc.scalar.activation(hn[qi][m][:], ps[qi][:, :QCH], AF.Relu,
                                                         bias=gbt[:, l, m:m + 1])
                        h = hn
                    for qi, q in enumerate(qpair):
                        for m in range(2):
                            nc.vector.reduce_sum(xf[:, m, q * 5:(q + 1) * 5],
                                                 h[qi][m].rearrange("p (b e) -> p b e", e=81), axis=AX.X)

                # ---- fbn stats + allreduce ----
                fst = ph3.tile([128, 4], F32, tag="fst")
                sqf = ph3.tile([128, 150], F32, tag="sqf")
                for m in range(2):
                    nc.vector.reduce_sum(fst[:, 2 * m:2 * m + 1], xf[:, m], axis=AX.X)
                    nc.scalar.activation(sqf[:], xf[:, m], AF.Square,
                                         accum_out=fst[:, 2 * m + 1:2 * m + 2])
                fbin = dram.tile([128, 4], F32, tag="ccfin")
                fbout = dram.tile([128 * n_cores, 4], F32, tag="ccfout")
                nc.sync.dma_start(fbin[:], fst[:])
                nc.gpsimd.collective_compute("AllGather", ALU.bypass, replica_groups=RG,
                                             ins=[fbin.opt()], outs=[fbout.opt()])
                fgat = ph3.tile([128, 4 * n_cores], F32, tag="fgat")
                nc.sync.dma_start(fgat[:], fbout.rearrange("(r p) f -> p r f", p=128))
                fred = ph3.tile([128, 4], F32, tag="fred")
                nc.vector.reduce_sum(fred[:], fgat.rearrange("p (r f) -> p f r", r=n_cores),
                                     axis=AX.X)
                fsc = ph3.tile([128, 2], F32, tag="fsc")
                fsh = ph3.tile([128, 2], F32, tag="fsh")
                for m in range(2):
                    _bn_scalar_ops(nc, ph3, fred[:, 2 * m:2 * m + 1], fred[:, 2 * m + 1:2 * m + 2],
                                   1200.0, fbng[:, m:m + 1], fbnb[:, m:m + 1],
                                   fsc[:, m:m + 1], fsh[:, m:m + 1], epsc[:], f"f{m}")

                if debug:
                    nc.sync.dma_start(xf_dbg_d[:], xf[:])

                # ---- f-MLP on [*, 150] ----
                y = [ph3.tile([128, 150], BF16, tag=f"y{m}", name=f"y{m}") for m in range(2)]
                for m in range(2):
                    nc.scalar.activation(y[m][:], xf[:, m], AF.Identity,
                                         bias=fsh[:, m:m + 1], scale=fsc[:, m:m + 1])
                for l in range(2):
                    yn = [ph3.tile([128, 150], BF16, tag=f"yn{l}_{m}", name=f"yn{l}_{m}") for m in range(2)]
                    for m in range(2):
                        ps = psg.tile([128, 512], F32, tag=f"gps{m}")
                        nc.tensor.matmul(ps[:, :150], fwt[:, l, 0, m * 128:(m + 1) * 128], y[0][:],
                                         start=True, stop=False)
                        nc.tensor.matmul(ps[:, :150], fwt[:, l, 1, m * 128:(m + 1) * 128], y[1][:],
                                         start=False, stop=True)
                        nc.scalar.activation(yn[m][:], ps[:, :150], AF.Relu, bias=fbt[:, l, m:m + 1])
                    y = yn
                z3 = ph3.tile([64, 150], BF16, tag="z3")
                ps = psg.tile([128, 512], F32, tag="gps0")
                nc.tensor.matmul(ps[0:64, :150], fw3[:, 0, :], y[0][:], start=True, stop=False)
                nc.tensor.matmul(ps[0:64, :150], fw3[:, 1, :], y[1][:], start=False, stop=True)
                nc.scalar.activation(z3[:], ps[0:64, :150], AF.Relu, bias=fb3[:, 0:1])
                ps4 = psg.tile([128, 512], F32, tag="gps1")
                nc.tensor.matmul(ps4[0:1, :150], fw4[:, 0:1], z3[:])
                score = ph3.tile([1, 150], F32, tag="score")
                nc.scalar.activation(score[:], ps4[0:1, :150], AF.Sigmoid, bias=fb4[0:1, 0:1])
                dist = ph3.tile([1, 150], F32, tag="dist")
                nc.vector.tensor_scalar(dist[:], score[:], -1.0, 1.0, ALU.mult, ALU.add)
                if debug:
                    nc.sync.dma_start(dist_dbg_d[:], dist[:])

                # ---- margin loss (exact sorted(label*dist)[1] semantics) ----
                v = ph3.tile([1, 150], F32, tag="lv0")
                nc.vector.tensor_tensor(v[:], dist[:], lbl_sb[:], ALU.mult)
                vq = v.rearrange("p (q s) -> p q s", s=S)
                min1 = ph3.tile([1, 30], F32, tag="min1")
                nc.vector.tensor_reduce(min1[:], vq, axis=AX.X, op=ALU.min)
                eq = ph3.tile([1, 150], F32, tag="eq")
                nc.vector.tensor_tensor(eq.rearrange("p (q s) -> p q s", s=S), vq,
                                        min1[:, :, None].to_broadcast((1, 30, 5)), ALU.is_equal)
                cntg = ph3.tile([1, 30], F32, tag="cntg")  # 1.0 if >=2 mins tie
                nc.vector.reduce_sum(cntg[:], eq.rearrange("p (q s) -> p q s", s=S), axis=AX.X)
                nc.vector.tensor_scalar(cntg[:], cntg[:], 1.5, None, ALU.is_ge)
                vx = ph3.tile([1, 150], F32, tag="vx")
                nc.vector.scalar_tensor_tensor(vx[:], eq[:], 1e9, v[:], ALU.mult, ALU.add)
                excl = ph3.tile([1, 30], F32, tag="excl")
                nc.vector.tensor_reduce(excl[:], vx.rearrange("p (q s) -> p q s", s=S),
                                        axis=AX.X, op=ALU.min)
                # min_neg = cntg ? min1 : excl
                nsel = ph3.tile([1, 30], F32, tag="nsel")
                nc.vector.tensor_scalar(nsel[:], cntg[:], -1.0, 1.0, ALU.mult, ALU.add)
                mn = ph3.tile([1, 30], F32, tag="mn")
                nc.vector.tensor_tensor(mn[:], min1[:], cntg[:], ALU.mult)
                nc.vector.tensor_tensor(nsel[:], excl[:], nsel[:], ALU.mult)
                nc.vector.tensor_tensor(mn[:], mn[:], nsel[:], ALU.add)
                t2 = ph3.tile([1, 150], F32, tag="lt2")
                nc.vector.tensor_tensor(t2[:], dist[:], apmask_sb[:], ALU.mult)
                ap_ = ph3.tile([1, 30], F32, tag="ap")
                nc.vector.reduce_sum(ap_[:], t2.rearrange("p (q s) -> p q s", s=S), axis=AX.X)
                dd = ph3.tile([1, 30], F32, tag="dd")
                nc.vector.tensor_tensor(dd[:], ap_[:], mn[:], ALU.subtract)
                lv = ph3.tile([1, 30], F32, tag="lv")
                nc.vector.tensor_scalar(lv[:], dd[:], margin[0:1, 0:1], 0.0, ALU.add, ALU.max)
                lp = ph3.tile([1, 1], F32, tag="lp")
                nc.vector.reduce_sum(lp[:], lv[:], axis=AX.X)
                nc.sync.dma_start(loss_d[:], lp[:])

    nc.compile()
    return nc


# ---------------------------------------------------------------------------
# host-side preparation
# ---------------------------------------------------------------------------

def _coord():
    ii = np.arange(3, dtype=np.float32) / 3.0
    c = np.stack([np.broadcast_to(ii[:, None], (3, 3)),
                  np.broadcast_to(ii[None, :], (3, 3))], 0).reshape(2, 9)
    return c


def make_in_maps(inp, n_cores=NCORES):
    p = {k: np.ascontiguousarray(np.asarray(v)) for k, v in inp.items()}
    coord = _coord()
    shared = {}
    w27 = p["w1"].transpose(2, 3, 1, 0).reshape(27, 64).astype(np.float32)
    w1t = np.zeros((54, 128), np.float32)
    w1t[0:27, 0:64] = w27; w1t[27:54, 64:128] = w27
    shared["w1t"] = w1t.astype(ml_dtypes.bfloat16)
    wct = np.stack([p["w2"], p["w3"], p["w4"]]).transpose(0, 3, 4, 2, 1).reshape(3, 9, 64, 64)
    wct = wct.transpose(2, 0, 1, 3)  # [ci, l, j, co]
    wbd = np.zeros((128, 3, 9, 128), np.float32)
    wbd[0:64, :, :, 0:64] = wct
    wbd[64:128, :, :, 64:128] = wct
    shared["wct"] = wbd
    bng = np.stack([p[f"bn{i}_g"] for i in range(1, 5)], 1).astype(np.float32)
    bnb = np.stack([p[f"bn{i}_b"] for i in range(1, 5)], 1).astype(np.float32)
    shared["bnx"] = np.concatenate([bng, bnb], 1)
    shared["gw1"] = np.concatenate([p["gw1"][:66], p["gw1"][66:]], 1).astype(np.float32)
    gwt = np.stack([p["gw2"], p["gw3"], p["gw4"]]).reshape(3, 2, 128, 256).transpose(2, 0, 1, 3).reshape(128, 1536)
    fwt = np.stack([p["fw1"], p["fw2"]]).reshape(2, 2, 128, 256).transpose(2, 0, 1, 3).reshape(128, 1024)
    fw3 = p["fw3"].reshape(2, 128, 64).transpose(1, 0, 2).reshape(128, 128)
    shared["wbf"] = np.concatenate([gwt, fwt, fw3], 1).astype(ml_dtypes.bfloat16)
    gb1 = p["gb1"].reshape(2, 128).T
    gbt = np.stack([p["gb2"], p["gb3"], p["gb4"]]).reshape(3, 2, 128).transpose(2, 0, 1).reshape(128, 6)
    fbt = np.stack([p["fb1"], p["fb2"]]).reshape(2, 2, 128).transpose(2, 0, 1).reshape(128, 4)
    fbng = p["fbn_g"].reshape(2, 128).T
    fbnb = p["fbn_b"].reshape(2, 128).T
    shared["pk128"] = np.concatenate([gb1, gbt, fbt, fbng, fbnb], 1).astype(np.float32)
    shared["fb3t"] = p["fb3"].reshape(64, 1).astype(np.float32)
    shared["fw4t"] = p["fw4"].reshape(64, 1).astype(ml_dtypes.bfloat16)
    shared["fb4t"] = p["fb4"].reshape(1, 1).astype(np.float32)
    shared["coordp"] = np.concatenate([np.tile(coord, (1, 5)), np.tile(coord, (1, 30))], 1).astype(np.float32)

    in_maps = []
    for c in range(n_cores):
        m = dict(shared)
        sup, qry = p["support_x"][c], p["query_x"][c]
        order = [sup[i] for i in range(5)] + [qry[i] for i in range(13)] \
            + [qry[13 + i] for i in range(17)] + [np.zeros((3, 84, 84), np.float32)]
        imgs = np.zeros((36, 3, PLANE), np.float32)
        imgs[:, :, :7056] = np.stack(order).reshape(36, 3, 7056)
        m["imgs"] = imgs.astype(ml_dtypes.bfloat16)
        same = (p["support_y"][c][None, :] == p["query_y"][c][:, None])
        lbl = (~same).astype(np.float32).reshape(1, 150)
        pos_idx = np.argmax(same, axis=1)
        apm = np.zeros((Q, S), np.float32)
        apm[np.arange(Q), pos_idx] = 1.0
        m["lap"] = np.concatenate([lbl, apm.reshape(1, 150)], 1)
        in_maps.append(m)
    return in_maps


_NC_CACHE = {}


def kernel(**inputs) -> np.ndarray:
    key = (NCORES, False)
    if key not in _NC_CACHE:
        _NC_CACHE[key] = build_nc(NCORES, debug=False)
    nc = _NC_CACHE[key]
    in_maps = make_in_maps(inputs, NCORES)
    res = run_bass_kernel_spmd(nc, in_maps, core_ids=list(range(NCORES)),
                               trace=bool(int(os.environ.get("KTRACE", "0"))))
    if res.exec_time_ns is not None:
        print(f"HW exec time: {res.exec_time_ns} ns")
    total = np.float64(sum(np.float64(r["loss_part"][0, 0]) for r in res.results))
    return np.asarray(total / NCORES, dtype=np.float32)


if __name__ == "__main__":
    d = np.load("/root/problem/ref_inputs.npz")
    inp = {k: d[k] for k in d.files}
    out = kernel(**inp)
    ref = np.load("/root/problem/ref_out.npy")
    print("kernel:", out, "ref:", ref, "rel err:", abs(out - ref) / max(abs(ref), 1e-12))

